# revision 10
# baseline (speedup 1.0000x reference)
"""Entity-knowledge embedding lookup kernel for Trainium2 (8 NeuronCores).

Math: for each token t (B*L=4096 total) with 8 labels,
    y[t] = conv_w @ mean_{j,k}(fact_table[label[t,j]] viewed as [16,300]) + conv_b
The mean over the 128 (8 labels x 16 subvectors) rows commutes with the 1x1
conv, so the kernel is: gather+sum 8 fact rows per token (DMA CCE add),
tree-reduce 4800->300 on DVE, then a tiny matmul per 128-token group.

Sharding: data-parallel over tokens — 512 tokens per core; fact table and
conv weights replicated.
"""

import sys

import numpy as np

sys.path.insert(0, "/opt/trn_rl_repo")

import concourse.bacc as bacc
import concourse.bass as bass
import concourse.mybir as mybir
from concourse.masks import make_identity
from concourse.tile import TileContext

VOCAB = 20000
TOPK = 8
GLOVE = 300
OUTC = 100
B, L, NL = 32, 128, 8
NCORES = 8
TOKENS = B * L            # 4096
TPC = TOKENS // NCORES    # 512 tokens per core
GROUP = 128               # tokens per SBUF tile group
NGROUPS = TPC // GROUP    # 4
ROW = 2 * TOPK * GLOVE    # 4800 floats per fact row

F32 = mybir.dt.float32
I32 = mybir.dt.int32


def build_nc(loops=1):
    nc = bacc.Bacc("TRN2", target_bir_lowering=False, debug=False)

    fact = nc.dram_tensor("fact", [VOCAB, ROW], F32, kind="ExternalInput").ap()
    labels = nc.dram_tensor("labels", [TPC, NL], I32, kind="ExternalInput").ap()
    # conv_w.T pre-scaled by 1/128 on host: [300, 100]
    wt = nc.dram_tensor("wt", [GLOVE, OUTC], F32, kind="ExternalInput").ap()
    bias = nc.dram_tensor("bias", [OUTC, 1], F32, kind="ExternalInput").ap()
    # output transposed: [100, 512]; host transposes back
    y = nc.dram_tensor("y", [OUTC, TPC], F32, kind="ExternalOutput").ap()

    with TileContext(nc) as tc:
        with (
            tc.tile_pool(name="const", bufs=1) as cpool,
            tc.tile_pool(name="acc", bufs=4) as apool,
            tc.tile_pool(name="small", bufs=4) as spool,
            tc.tile_pool(name="ps_t", bufs=3, space="PSUM") as ppool_t,
            tc.tile_pool(name="ps_y", bufs=2, space="PSUM") as ppool_y,
        ):
            # constants are DVE-copied so PE instructions depend only on the
            # DVE semaphore (PE allows a single sync-wait slot on TRN2)
            ident0 = cpool.tile([128, 128], F32, tag="ident0")
            make_identity(nc, ident0[:])
            ident = cpool.tile([128, 128], F32, tag="ident")
            nc.vector.tensor_copy(ident[:], ident0[:])
            wts = []
            for k in range(3):
                t0 = cpool.tile([100, OUTC], F32, tag=f"wt{k}raw")
                nc.sync.dma_start(out=t0[:], in_=wt[k * 100 : (k + 1) * 100, :])
                t = cpool.tile([100, OUTC], F32, tag=f"wt{k}")
                nc.vector.tensor_copy(t[:], t0[:])
                wts.append(t)
            btile = cpool.tile([OUTC, 1], F32)
            nc.sync.dma_start(out=btile[:], in_=bias[:])

            for g in range(NGROUPS * loops):
                g = g % NGROUPS
                tok0 = g * GROUP
                idx = spool.tile([GROUP, NL], I32, tag="idx")
                nc.sync.dma_start(out=idx[:], in_=labels[tok0 : tok0 + GROUP, :])

                # accumulate 8 gathered fact rows per token via DMA CCE add.
                # CCE ops max out at 2048 elements per descriptor, so the
                # accumulating gathers are split into 1600-element chunks;
                # the first (bypass) gather can take the full row.
                acc = apool.tile([GROUP, ROW], F32, tag="acc")
                nc.gpsimd.indirect_dma_start(
                    out=acc[:],
                    out_offset=None,
                    in_=fact[:],
                    in_offset=bass.IndirectOffsetOnAxis(ap=idx[:, 0:1], axis=0),
                    compute_op=mybir.AluOpType.bypass,
                )
                CH = 1600
                for j in range(1, NL):
                    for k in range(ROW // CH):
                        nc.gpsimd.indirect_dma_start(
                            out=acc[:, k * CH : (k + 1) * CH],
                            out_offset=None,
                            in_=fact[:],
                            in_offset=bass.IndirectOffsetOnAxis(
                                ap=idx[:, j : j + 1], axis=0
                            ),
                            element_offset=k * CH,
                            compute_op=mybir.AluOpType.add,
                        )

                # tree-reduce free dim 4800 -> 600 in place, final step into a
                # fresh DVE-only tile (keeps PE transpose at <=2 sem waits)
                w = ROW
                while w > 2 * GLOVE:
                    h = w // 2
                    nc.vector.tensor_add(acc[:, :h], acc[:, :h], acc[:, h:w])
                    w = h
                s = spool.tile([GROUP, GLOVE], F32, tag="s")
                nc.vector.tensor_add(s[:], acc[:, :GLOVE], acc[:, GLOVE : 2 * GLOVE])

                # conv: out[o, t] = sum_c w[o,c]/128 * s[t, c]  (3 c-chunks of 100)
                yt = spool.tile([OUTC, GROUP], F32, tag="yt")
                nc.vector.tensor_copy(yt[:], btile[:].to_broadcast([OUTC, GROUP]))
                for k in range(3):
                    tp = ppool_t.tile([100, GROUP], F32, tag="tp")
                    nc.tensor.transpose(
                        out=tp[:], in_=s[:, k * 100 : (k + 1) * 100], identity=ident[:]
                    )
                    st = spool.tile([100, GROUP], F32, tag="st")
                    nc.vector.tensor_copy(st[:], tp[:])
                    yp = ppool_y.tile([OUTC, GROUP], F32, tag="yp")
                    nc.tensor.matmul(yp[:], wts[k][:], st[:], start=True, stop=True)
                    nc.vector.tensor_add(yt[:], yt[:], yp[:])
                nc.sync.dma_start(out=y[:, tok0 : tok0 + GROUP], in_=yt[:])

    nc.finalize()
    return nc


def make_in_maps(detect_labels, fact_table, conv_w, conv_b):
    labels_flat = np.ascontiguousarray(
        detect_labels.reshape(TOKENS, NL).astype(np.int32)
    )
    fact2d = np.ascontiguousarray(fact_table.reshape(VOCAB, ROW).astype(np.float32))
    wt = np.ascontiguousarray(conv_w.T.astype(np.float32) / 128.0)
    bias2d = np.ascontiguousarray(conv_b.astype(np.float32).reshape(OUTC, 1))
    in_maps = []
    for c in range(NCORES):
        in_maps.append(
            {
                "fact": fact2d,
                "labels": np.ascontiguousarray(labels_flat[c * TPC : (c + 1) * TPC]),
                "wt": wt,
                "bias": bias2d,
            }
        )
    return in_maps


def assemble_output(results):
    # results: list of per-core dicts with "y" [100, 512]
    parts = [np.asarray(r["y"]).T for r in results]  # each [512, 100]
    return np.concatenate(parts, axis=0).reshape(B, L, OUTC).astype(np.float32)


def kernel(detect_labels, fact_table, conv_w, conv_b):
    from concourse import bass_utils

    nc = build_nc()
    in_maps = make_in_maps(detect_labels, fact_table, conv_w, conv_b)
    res = bass_utils.run_bass_kernel_spmd(nc, in_maps, list(range(NCORES)))
    return assemble_output(res.results)


# revision 17
# speedup vs baseline: 18.2510x; 18.2510x over previous
"""Entity-knowledge embedding lookup kernel for Trainium2 (8 NeuronCores).

Math: for each token t (B*L=4096 total) with 8 labels,
    y[t] = conv_w @ mean_{j,k}(fact_table[label[t,j]] viewed as [16,300]) + conv_b
The mean over the 128 (8 labels x 16 subvectors) rows commutes with the 1x1
conv, so the kernel is: gather+sum 8 fact rows per token (DMA CCE add),
tree-reduce 4800->300 on DVE, then a tiny matmul per 128-token group.

Sharding: data-parallel over tokens — 512 tokens per core; fact table and
conv weights replicated.
"""

import sys

import numpy as np

sys.path.insert(0, "/opt/trn_rl_repo")

import concourse.bacc as bacc
import concourse.bass as bass
import concourse.mybir as mybir
from concourse.masks import make_identity
from concourse.tile import TileContext

VOCAB = 20000
TOPK = 8
GLOVE = 300
OUTC = 100
B, L, NL = 32, 128, 8
NCORES = 8
TOKENS = B * L            # 4096
TPC = TOKENS // NCORES    # 512 tokens per core
GROUP = 128               # tokens per SBUF tile group
NGROUPS = TPC // GROUP    # 4
ROW = 2 * TOPK * GLOVE    # 4800 floats per fact row
NCHUNK = 3                # CCE add maxes at 2048 elems; 4800/3 = 1600
CH = ROW // NCHUNK        # 1600 elements per gather chunk

F32 = mybir.dt.float32
I32 = mybir.dt.int32


def build_nc(loops=1):
    nc = bacc.Bacc("TRN2", target_bir_lowering=False, debug=False)

    fact = nc.dram_tensor("fact", [VOCAB, ROW], F32, kind="ExternalInput").ap()
    labels = nc.dram_tensor("labels", [TPC, NL], I32, kind="ExternalInput").ap()
    # conv_w.T pre-scaled by 1/128 on host: [300, 100]
    wt = nc.dram_tensor("wt", [GLOVE, OUTC], F32, kind="ExternalInput").ap()
    bias = nc.dram_tensor("bias", [OUTC, 1], F32, kind="ExternalInput").ap()
    # output transposed: [100, 512]; host transposes back
    y = nc.dram_tensor("y", [OUTC, TPC], F32, kind="ExternalOutput").ap()

    with TileContext(nc) as tc:
        with (
            tc.tile_pool(name="const", bufs=1) as cpool,
            tc.tile_pool(name="acc", bufs=4) as apool,
            tc.tile_pool(name="small", bufs=4) as spool,
            tc.tile_pool(name="ps_t", bufs=3, space="PSUM") as ppool_t,
            tc.tile_pool(name="ps_y", bufs=2, space="PSUM") as ppool_y,
        ):
            # constants are DVE-copied so PE instructions depend only on the
            # DVE semaphore (PE allows a single sync-wait slot on TRN2)
            ident0 = cpool.tile([128, 128], F32, tag="ident0")
            make_identity(nc, ident0[:])
            ident = cpool.tile([128, 128], F32, tag="ident")
            nc.vector.tensor_copy(ident[:], ident0[:])
            wts = []
            for k in range(3):
                t0 = cpool.tile([100, OUTC], F32, tag=f"wt{k}raw")
                nc.sync.dma_start(out=t0[:], in_=wt[k * 100 : (k + 1) * 100, :])
                t = cpool.tile([100, OUTC], F32, tag=f"wt{k}")
                nc.vector.tensor_copy(t[:], t0[:])
                wts.append(t)
            btile = cpool.tile([OUTC, 1], F32)
            nc.sync.dma_start(out=btile[:], in_=bias[:])

            for _ in range(loops):
                # phase A: per group, load indices + bypass gather (j=0);
                # phase B: accumulating gathers interleaved j-outer so the
                # in-order SWDGE queue never stalls on a dependent chain link
                idxs, accs = [], []
                for g in range(NGROUPS):
                    tok0 = g * GROUP
                    idx = spool.tile([GROUP, NL], I32, tag="idx")
                    nc.sync.dma_start(out=idx[:], in_=labels[tok0 : tok0 + GROUP, :])
                    idxs.append(idx)
                    acc = apool.tile([GROUP, ROW], F32, tag="acc")
                    nc.gpsimd.indirect_dma_start(
                        out=acc[:],
                        out_offset=None,
                        in_=fact[:],
                        in_offset=bass.IndirectOffsetOnAxis(ap=idx[:, 0:1], axis=0),
                        compute_op=mybir.AluOpType.bypass,
                    )
                    accs.append(acc)
                # CCE add maxes at 2048 elements per descriptor -> 3 chunks
                for j in range(1, NL):
                    for g in range(NGROUPS):
                        for k in range(NCHUNK):
                            nc.gpsimd.indirect_dma_start(
                                out=accs[g][:, k * CH : (k + 1) * CH],
                                out_offset=None,
                                in_=fact[:],
                                in_offset=bass.IndirectOffsetOnAxis(
                                    ap=idxs[g][:, j : j + 1], axis=0
                                ),
                                element_offset=k * CH,
                                compute_op=mybir.AluOpType.add,
                            )

                for g in range(NGROUPS):
                    acc = accs[g]
                    tok0 = g * GROUP
                    # tree-reduce free dim 4800 -> 600 in place, final step
                    # into a fresh DVE-only tile (keeps PE at 1 sem wait)
                    w = ROW
                    while w > 2 * GLOVE:
                        h = w // 2
                        nc.vector.tensor_add(acc[:, :h], acc[:, :h], acc[:, h:w])
                        w = h
                    s = spool.tile([GROUP, GLOVE], F32, tag="s")
                    nc.vector.tensor_add(
                        s[:], acc[:, :GLOVE], acc[:, GLOVE : 2 * GLOVE]
                    )

                    # conv: out[o,t] = sum_c w[o,c]/128 * s[t,c] (3 c-chunks)
                    yt = spool.tile([OUTC, GROUP], F32, tag="yt")
                    nc.vector.tensor_copy(yt[:], btile[:].to_broadcast([OUTC, GROUP]))
                    for k in range(3):
                        tp = ppool_t.tile([100, GROUP], F32, tag="tp")
                        nc.tensor.transpose(
                            out=tp[:],
                            in_=s[:, k * 100 : (k + 1) * 100],
                            identity=ident[:],
                        )
                        st = spool.tile([100, GROUP], F32, tag="st")
                        nc.vector.tensor_copy(st[:], tp[:])
                        yp = ppool_y.tile([OUTC, GROUP], F32, tag="yp")
                        nc.tensor.matmul(
                            yp[:], wts[k][:], st[:], start=True, stop=True
                        )
                        nc.vector.tensor_add(yt[:], yt[:], yp[:])
                    nc.sync.dma_start(out=y[:, tok0 : tok0 + GROUP], in_=yt[:])

    nc.finalize()
    return nc


def make_in_maps(detect_labels, fact_table, conv_w, conv_b):
    labels_flat = np.ascontiguousarray(
        detect_labels.reshape(TOKENS, NL).astype(np.int32)
    )
    fact2d = np.ascontiguousarray(fact_table.reshape(VOCAB, ROW).astype(np.float32))
    wt = np.ascontiguousarray(conv_w.T.astype(np.float32) / 128.0)
    bias2d = np.ascontiguousarray(conv_b.astype(np.float32).reshape(OUTC, 1))
    in_maps = []
    for c in range(NCORES):
        in_maps.append(
            {
                "fact": fact2d,
                "labels": np.ascontiguousarray(labels_flat[c * TPC : (c + 1) * TPC]),
                "wt": wt,
                "bias": bias2d,
            }
        )
    return in_maps


def assemble_output(results):
    # results: list of per-core dicts with "y" [100, 512]
    parts = [np.asarray(r["y"]).T for r in results]  # each [512, 100]
    return np.concatenate(parts, axis=0).reshape(B, L, OUTC).astype(np.float32)


def kernel(detect_labels, fact_table, conv_w, conv_b):
    from concourse import bass_utils

    nc = build_nc()
    in_maps = make_in_maps(detect_labels, fact_table, conv_w, conv_b)
    res = bass_utils.run_bass_kernel_spmd(nc, in_maps, list(range(NCORES)))
    return assemble_output(res.results)


# revision 19
# speedup vs baseline: 49.4562x; 2.7098x over previous
"""Entity-knowledge embedding lookup kernel for Trainium2 (8 NeuronCores).

Math: for each token t (B*L=4096 total) with 8 labels,
    y[t] = conv_w @ mean_{j,k}(fact_table[label[t,j]] viewed as [16,300]) + conv_b
The mean over the 128 (8 labels x 16 subvectors) rows commutes with the 1x1
conv, so the kernel is: gather+sum 8 fact rows per token (DMA CCE add),
tree-reduce 4800->300 on DVE, then a tiny matmul per 128-token group.

Sharding: data-parallel over tokens — 512 tokens per core; fact table and
conv weights replicated.
"""

import sys

import numpy as np

sys.path.insert(0, "/opt/trn_rl_repo")

import concourse.bacc as bacc
import concourse.bass as bass
import concourse.mybir as mybir
from concourse.masks import make_identity
from concourse.tile import TileContext

VOCAB = 20000
TOPK = 8
GLOVE = 300
OUTC = 100
B, L, NL = 32, 128, 8
NCORES = 8
TOKENS = B * L            # 4096
TPC = TOKENS // NCORES    # 512 tokens per core
GROUP = 128               # tokens per SBUF tile group
NGROUPS = TPC // GROUP    # 4
ROW = 2 * TOPK * GLOVE    # 4800 floats per fact row
NCHUNK = 3                # CCE add maxes at 2048 elems; 4800/3 = 1600
CH = ROW // NCHUNK        # 1600 elements per gather chunk

F32 = mybir.dt.float32
I32 = mybir.dt.int32


def build_nc(loops=1):
    nc = bacc.Bacc("TRN2", target_bir_lowering=False, debug=False)

    fact = nc.dram_tensor("fact", [VOCAB, ROW], F32, kind="ExternalInput").ap()
    labels = nc.dram_tensor("labels", [TPC, NL], I32, kind="ExternalInput").ap()
    # conv_w.T pre-scaled by 1/128 on host: [300, 100]
    wt = nc.dram_tensor("wt", [GLOVE, OUTC], F32, kind="ExternalInput").ap()
    bias = nc.dram_tensor("bias", [OUTC, 1], F32, kind="ExternalInput").ap()
    # output transposed: [100, 512]; host transposes back
    y = nc.dram_tensor("y", [OUTC, TPC], F32, kind="ExternalOutput").ap()

    with TileContext(nc) as tc:
        with (
            tc.tile_pool(name="const", bufs=1) as cpool,
            tc.tile_pool(name="acc", bufs=4) as apool,
            tc.tile_pool(name="small", bufs=4) as spool,
            tc.tile_pool(name="ps_t", bufs=3, space="PSUM") as ppool_t,
            tc.tile_pool(name="ps_y", bufs=2, space="PSUM") as ppool_y,
        ):
            # constants are DVE-copied so PE instructions depend only on the
            # DVE semaphore (PE allows a single sync-wait slot on TRN2)
            ident0 = cpool.tile([128, 128], F32, tag="ident0")
            make_identity(nc, ident0[:])
            ident = cpool.tile([128, 128], F32, tag="ident")
            nc.vector.tensor_copy(ident[:], ident0[:])
            wts = []
            for k in range(3):
                t0 = cpool.tile([100, OUTC], F32, tag=f"wt{k}raw")
                nc.sync.dma_start(out=t0[:], in_=wt[k * 100 : (k + 1) * 100, :])
                t = cpool.tile([100, OUTC], F32, tag=f"wt{k}")
                nc.vector.tensor_copy(t[:], t0[:])
                wts.append(t)
            btile = cpool.tile([OUTC, 1], F32)
            nc.sync.dma_start(out=btile[:], in_=bias[:])

            for _ in range(loops):
                # phase A: per group, load indices + bypass gather (j=0);
                # phase B: accumulating gathers interleaved j-outer so the
                # in-order SWDGE queue never stalls on a dependent chain link
                idxs, accs = [], []
                for g in range(NGROUPS):
                    tok0 = g * GROUP
                    idx = spool.tile([GROUP, NL], I32, tag="idx")
                    nc.sync.dma_start(out=idx[:], in_=labels[tok0 : tok0 + GROUP, :])
                    idxs.append(idx)
                    acc = apool.tile([GROUP, ROW], F32, tag="acc")
                    nc.gpsimd.indirect_dma_start(
                        out=acc[:],
                        out_offset=None,
                        in_=fact[:],
                        in_offset=bass.IndirectOffsetOnAxis(ap=idx[:, 0:1], axis=0),
                        compute_op=mybir.AluOpType.bypass,
                    )
                    accs.append(acc)
                # CCE add maxes at 2048 elements per descriptor -> 3 chunks
                for j in range(1, NL):
                    for g in range(NGROUPS):
                        for k in range(NCHUNK):
                            nc.gpsimd.indirect_dma_start(
                                out=accs[g][:, k * CH : (k + 1) * CH],
                                out_offset=None,
                                in_=fact[:],
                                in_offset=bass.IndirectOffsetOnAxis(
                                    ap=idxs[g][:, j : j + 1], axis=0
                                ),
                                element_offset=k * CH,
                                compute_op=mybir.AluOpType.add,
                            )

                for g in range(NGROUPS):
                    acc = accs[g]
                    tok0 = g * GROUP
                    # tree-reduce free dim 4800 -> 600 in place, final step
                    # into a fresh DVE-only tile (keeps PE at 1 sem wait)
                    w = ROW
                    while w > 2 * GLOVE:
                        h = w // 2
                        nc.vector.tensor_add(acc[:, :h], acc[:, :h], acc[:, h:w])
                        w = h
                    s = spool.tile([GROUP, GLOVE], F32, tag="s")
                    nc.vector.tensor_add(
                        s[:], acc[:, :GLOVE], acc[:, GLOVE : 2 * GLOVE]
                    )

                    # conv: out[o,t] = sum_c w[o,c]/128 * s[t,c] (3 c-chunks)
                    yt = spool.tile([OUTC, GROUP], F32, tag="yt")
                    nc.vector.tensor_copy(yt[:], btile[:].to_broadcast([OUTC, GROUP]))
                    for k in range(3):
                        tp = ppool_t.tile([100, GROUP], F32, tag="tp")
                        nc.tensor.transpose(
                            out=tp[:],
                            in_=s[:, k * 100 : (k + 1) * 100],
                            identity=ident[:],
                        )
                        st = spool.tile([100, GROUP], F32, tag="st")
                        nc.vector.tensor_copy(st[:], tp[:])
                        yp = ppool_y.tile([OUTC, GROUP], F32, tag="yp")
                        nc.tensor.matmul(
                            yp[:], wts[k][:], st[:], start=True, stop=True
                        )
                        nc.vector.tensor_add(yt[:], yt[:], yp[:])
                    nc.sync.dma_start(out=y[:, tok0 : tok0 + GROUP], in_=yt[:])

    nc.finalize()
    return nc


def build_nc_bypass(loops=1):
    """All-bypass variant: per (group, label) gather [128 tokens, 4800] with
    plain bypass (no DMA CCE), DVE tree-reduce each to [128, 300], and
    DVE-accumulate the 8 labels into the group sum. Same inputs as v1."""
    nc = bacc.Bacc("TRN2", target_bir_lowering=False, debug=False)

    fact = nc.dram_tensor("fact", [VOCAB, ROW], F32, kind="ExternalInput").ap()
    labels = nc.dram_tensor("labels", [TPC, NL], I32, kind="ExternalInput").ap()
    wt = nc.dram_tensor("wt", [GLOVE, OUTC], F32, kind="ExternalInput").ap()
    bias = nc.dram_tensor("bias", [OUTC, 1], F32, kind="ExternalInput").ap()
    y = nc.dram_tensor("y", [OUTC, TPC], F32, kind="ExternalOutput").ap()

    with TileContext(nc) as tc:
        with (
            tc.tile_pool(name="const", bufs=1) as cpool,
            tc.tile_pool(name="acc", bufs=6) as apool,
            tc.tile_pool(name="small", bufs=4) as spool,
            tc.tile_pool(name="ssum", bufs=3) as sspool,
            tc.tile_pool(name="ps_t", bufs=3, space="PSUM") as ppool_t,
            tc.tile_pool(name="ps_y", bufs=2, space="PSUM") as ppool_y,
        ):
            ident0 = cpool.tile([128, 128], F32, tag="ident0")
            make_identity(nc, ident0[:])
            ident = cpool.tile([128, 128], F32, tag="ident")
            nc.vector.tensor_copy(ident[:], ident0[:])
            wts = []
            for k in range(3):
                t0 = cpool.tile([100, OUTC], F32, tag=f"wt{k}raw")
                nc.sync.dma_start(out=t0[:], in_=wt[k * 100 : (k + 1) * 100, :])
                t = cpool.tile([100, OUTC], F32, tag=f"wt{k}")
                nc.vector.tensor_copy(t[:], t0[:])
                wts.append(t)
            btile = cpool.tile([OUTC, 1], F32)
            nc.sync.dma_start(out=btile[:], in_=bias[:])

            for _ in range(loops):
                for g in range(NGROUPS):
                    tok0 = g * GROUP
                    idx = spool.tile([GROUP, NL], I32, tag="idx")
                    nc.sync.dma_start(out=idx[:], in_=labels[tok0 : tok0 + GROUP, :])
                    ssum = sspool.tile([GROUP, GLOVE], F32, tag="ssum")
                    for j in range(NL):
                        acc = apool.tile([GROUP, ROW], F32, tag="acc")
                        nc.gpsimd.indirect_dma_start(
                            out=acc[:],
                            out_offset=None,
                            in_=fact[:],
                            in_offset=bass.IndirectOffsetOnAxis(
                                ap=idx[:, j : j + 1], axis=0
                            ),
                            compute_op=mybir.AluOpType.bypass,
                        )
                        w = ROW
                        while w > 2 * GLOVE:
                            h = w // 2
                            nc.vector.tensor_add(acc[:, :h], acc[:, :h], acc[:, h:w])
                            w = h
                        if j == 0:
                            nc.vector.tensor_add(
                                ssum[:], acc[:, :GLOVE], acc[:, GLOVE : 2 * GLOVE]
                            )
                        else:
                            sj = spool.tile([GROUP, GLOVE], F32, tag="sj")
                            nc.vector.tensor_add(
                                sj[:], acc[:, :GLOVE], acc[:, GLOVE : 2 * GLOVE]
                            )
                            nc.vector.tensor_add(ssum[:], ssum[:], sj[:])

                    yt = spool.tile([OUTC, GROUP], F32, tag="yt")
                    nc.vector.tensor_copy(yt[:], btile[:].to_broadcast([OUTC, GROUP]))
                    for k in range(3):
                        tp = ppool_t.tile([100, GROUP], F32, tag="tp")
                        nc.tensor.transpose(
                            out=tp[:],
                            in_=ssum[:, k * 100 : (k + 1) * 100],
                            identity=ident[:],
                        )
                        st = spool.tile([100, GROUP], F32, tag="st")
                        nc.vector.tensor_copy(st[:], tp[:])
                        yp = ppool_y.tile([OUTC, GROUP], F32, tag="yp")
                        nc.tensor.matmul(
                            yp[:], wts[k][:], st[:], start=True, stop=True
                        )
                        nc.vector.tensor_add(yt[:], yt[:], yp[:])
                    nc.sync.dma_start(out=y[:, tok0 : tok0 + GROUP], in_=yt[:])

    nc.finalize()
    return nc


def make_in_maps(detect_labels, fact_table, conv_w, conv_b):
    labels_flat = np.ascontiguousarray(
        detect_labels.reshape(TOKENS, NL).astype(np.int32)
    )
    fact2d = np.ascontiguousarray(fact_table.reshape(VOCAB, ROW).astype(np.float32))
    wt = np.ascontiguousarray(conv_w.T.astype(np.float32) / 128.0)
    bias2d = np.ascontiguousarray(conv_b.astype(np.float32).reshape(OUTC, 1))
    in_maps = []
    for c in range(NCORES):
        in_maps.append(
            {
                "fact": fact2d,
                "labels": np.ascontiguousarray(labels_flat[c * TPC : (c + 1) * TPC]),
                "wt": wt,
                "bias": bias2d,
            }
        )
    return in_maps


def assemble_output(results):
    # results: list of per-core dicts with "y" [100, 512]
    parts = [np.asarray(r["y"]).T for r in results]  # each [512, 100]
    return np.concatenate(parts, axis=0).reshape(B, L, OUTC).astype(np.float32)


def kernel(detect_labels, fact_table, conv_w, conv_b):
    from concourse import bass_utils

    nc = build_nc()
    in_maps = make_in_maps(detect_labels, fact_table, conv_w, conv_b)
    res = bass_utils.run_bass_kernel_spmd(nc, in_maps, list(range(NCORES)))
    return assemble_output(res.results)
